# revision 20
# baseline (speedup 1.0000x reference)
"""Trainium2 Bass kernel for nn_AttentionSampling (gnn_message_passing).

Math: X = l2norm(embeds); S = X@X.T + 0.5*(adj+I)@(X@X.T), diag=-inf;
top-32 per row + unique of indices.

Factored form used on device:  S = W @ X^T  with  W = 1.5*X + 0.5*(adj@X)
(16x fewer FLOPs than the naive 137-GFLOP form; identical math up to fp
rounding).

Sharding: row-shard across 8 cores (512 rows each).  Each core:
  - normalizes X (all 4096 rows) + its own 512 rows
  - Y^T = X^T @ adjT_local  (PE, fp32)
  - W^T = 1.5*Xl^T + 0.5*Y^T
  - S tiles = W^T.T @ X^T   (PE, fp32), streamed to DRAM; per-64-chunk max
  - selects the top-33(+ties) chunks per row via an exact chunk-max
    threshold (bitonic sorts on the 64 chunk maxima), gathers those
    chunks with a per-row indirect DMA
  - outputs gathered candidate chunks + sorted chunk keys
Host: decodes candidates (a guaranteed exact superset of the top-32),
masks the diagonal, takes the exact top-32 per row, and builds the
reference's (sampled_nodes, indices, values) tuple.
"""

import sys

for _p in ("/opt/trn_rl_repo", "/root/.axon_site/_ro/trn_rl_repo"):
    if _p not in sys.path:
        sys.path.insert(0, _p)

from contextlib import ExitStack

import numpy as np

import concourse.bass as bass
import concourse.bacc as bacc
import concourse.mybir as mybir
import concourse.tile as tile
from concourse.bass import IndirectOffsetOnAxis
from concourse.bass_utils import run_bass_kernel_spmd

F32 = mybir.dt.float32
I32 = mybir.dt.int32
X_AX = mybir.AxisListType.X
OP = mybir.AluOpType

import os as _os
MM_F32R = bool(_os.environ.get("MM_F32R"))
N, D = 4096, 128
NC = 8            # cores
RPC = N // NC     # rows per core = 512
TOPK = 32
CH = 64           # chunk size
NCHUNK = N // CH  # 64 chunks per row
NSLOT = 34        # gathered candidate chunks per row (33 + tie margin)
NEG = -3.0e38


def _bitonic_sort_desc(nc, pool, buf):
    """In-place descending bitonic sort along the last axis of buf
    ([128, G, n] fp32 SBUF tile AP). n must be a power of 2."""
    p, g, n = buf.shape
    scratch = pool.tile([p, g, n // 2], F32, tag="bsort_scr")
    k = 2
    while k <= n:
        j = k // 2
        while j >= 1:
            # windows of 2j; element i pairs with i+j.
            nw = n // (2 * j)          # windows per row
            wk = max(k // (2 * j), 1)  # windows per direction block
            v = buf.rearrange("p a (b w x t) -> p a b w x t", w=wk, x=2, t=j)
            nb = v.shape[2]            # direction blocks (alternate desc/asc)
            for par, desc in ((0, True), (1, False)) if nb > 1 else ((0, True),):
                blk = v[:, :, par::2] if nb > 1 else v
                lo = blk[:, :, :, :, 0, :]
                hi = blk[:, :, :, :, 1, :]
                cnt = int(np.prod(lo.shape[2:]))
                scr = scratch[:, :, :cnt].rearrange(
                    "p a (b w t) -> p a b w t", w=lo.shape[3], t=j
                )
                if desc:
                    nc.vector.tensor_tensor(scr, lo, hi, op=OP.min)
                    nc.vector.tensor_tensor(lo, lo, hi, op=OP.max)
                else:
                    nc.vector.tensor_tensor(scr, lo, hi, op=OP.max)
                    nc.vector.tensor_tensor(lo, lo, hi, op=OP.min)
                nc.vector.tensor_copy(hi, scr)
            j //= 2
        k *= 2


def build_bass():
    nc = bacc.Bacc(None)
    emb = nc.dram_tensor("emb", [N, D], F32, kind="ExternalInput").ap()
    adjt = nc.dram_tensor("adjt", [N, RPC], F32, kind="ExternalInput").ap()
    embsel = nc.dram_tensor("embsel", [RPC, D], F32, kind="ExternalInput").ap()
    eye = nc.dram_tensor("eye", [128, 128], F32, kind="ExternalInput").ap()
    dio3 = nc.dram_tensor("dio3", [128, 4, CH], F32, kind="ExternalInput").ap()
    rowb = nc.dram_tensor("rowb", [128, 4], F32, kind="ExternalInput").ap()

    out_g = nc.dram_tensor("out_g", [RPC, NSLOT * CH], F32, kind="ExternalOutput").ap()
    out_key = nc.dram_tensor("out_key", [RPC, NCHUNK], F32, kind="ExternalOutput").ap()

    s_dram = nc.dram_tensor("s_scratch", [RPC * NCHUNK, CH], F32, kind="Internal").ap()
    didx = nc.dram_tensor("didx_scratch", [4 * NSLOT * 8 * 128], mybir.dt.int16,
                          kind="Internal").ap()

    with TileKernel(nc) as body:
        body(emb, adjt, embsel, eye, dio3, rowb, out_g, out_key, s_dram, didx)
    nc.finalize()
    return nc


class TileKernel:
    def __init__(self, nc):
        self.nc = nc
        self.stack = ExitStack()

    def __enter__(self):
        self.tc = self.stack.enter_context(tile.TileContext(self.nc))
        return self._body

    def __exit__(self, *exc):
        return self.stack.__exit__(*exc)

    def _body(self, emb, adjt, embsel, eye, dio3, rowb, out_g, out_key, s_dram,
              didx):
        nc, tc, ctx = self.nc, self.tc, self.stack
        const = ctx.enter_context(tc.tile_pool(name="const", bufs=1))
        work = ctx.enter_context(tc.tile_pool(name="work", bufs=1))
        adjp = ctx.enter_context(tc.tile_pool(name="adjp", bufs=3))
        sstg = ctx.enter_context(tc.tile_pool(name="sstg", bufs=4))
        psA = ctx.enter_context(tc.tile_pool(name="psA", bufs=2, space="PSUM"))
        psY = ctx.enter_context(tc.tile_pool(name="psY", bufs=1, space="PSUM"))
        psS = ctx.enter_context(tc.tile_pool(name="psS", bufs=4, space="PSUM"))

        eye_t = const.tile([128, 128], F32)
        nc.gpsimd.dma_start(eye_t[:], eye)
        dio_t = const.tile([128, 4, CH], F32)
        nc.gpsimd.dma_start(dio_t[:], dio3)
        rowb_t = const.tile([128, 4], F32)
        nc.gpsimd.dma_start(rowb_t[:], rowb)

        # ---- load embeds (rows r=t*128+p on partition p, group t) ----
        embt = work.tile([128, N // 128, D], F32)
        nc.gpsimd.dma_start(embt[:], emb.rearrange("(t p) d -> p t d", p=128))
        eselt = work.tile([128, RPC // 128, D], F32)
        nc.gpsimd.dma_start(eselt[:], embsel.rearrange("(g p) d -> p g d", p=128))

        # ---- normalize: X = emb / sqrt(sum(emb^2)) ----
        def normalize(src, nt):
            sq = work.tile([128, nt, D], F32, tag="sq")
            nc.scalar.activation(sq[:], src[:],
                                 mybir.ActivationFunctionType.Square)
            ss = work.tile([128, nt], F32, tag="ss")
            nc.vector.reduce_sum(ss[:], sq[:], axis=X_AX)
            nrm = work.tile([128, nt], F32, tag="nrm")
            nc.scalar.activation(nrm[:], ss[:], mybir.ActivationFunctionType.Sqrt)
            inv = work.tile([128, nt], F32, tag="inv")
            nc.vector.reciprocal(inv[:], nrm[:])
            xo = work.tile([128, nt, D], F32, tag="xnorm" + str(nt))
            nc.vector.tensor_tensor(xo[:], src[:], inv[:].broadcast_to((128, nt, D)),
                                    op=OP.mult)
            return xo

        X = normalize(embt, N // 128)        # [128, 32, 128] rows x d
        Xl = normalize(eselt, RPC // 128)    # [128, 4, 128] own rows x d

        # ---- transposes via PE: X_T [128 d, 4096 rows], Xl_T [128 d, 512] ----
        xt = work.tile([128, N], F32)
        for t in range(N // 128):
            ps = psA.tile([128, 128], F32)
            nc.tensor.transpose(ps[:], X[:, t, :], eye_t[:])
            nc.scalar.copy(xt[:, t * 128:(t + 1) * 128], ps[:])
        xlt = work.tile([128, RPC], F32)
        for g in range(RPC // 128):
            ps = psA.tile([128, 128], F32)
            nc.tensor.transpose(ps[:], Xl[:, g, :], eye_t[:])
            nc.scalar.copy(xlt[:, g * 128:(g + 1) * 128], ps[:])

        # ---- Y^T = X^T @ adjT_local : accumulate over 32 k-chunks ----
        yps = psY.tile([128, RPC], F32)
        for kc in range(N // 128):
            at = adjp.tile([128, RPC], F32)
            nc.gpsimd.dma_start(at[:], adjt[kc * 128:(kc + 1) * 128, :])
            _l, _r = X[:, kc, :], at[:]
            if MM_F32R:
                _l, _r = _l.bitcast(mybir.dt.float32r), _r.bitcast(mybir.dt.float32r)
            nc.tensor.matmul(yps[:], _l, _r,
                             start=(kc == 0), stop=(kc == N // 128 - 1))

        # ---- W^T = 1.5*Xl^T + 0.5*Y^T ----
        wt = work.tile([128, RPC], F32)
        nc.vector.tensor_scalar_mul(wt[:], xlt[:], 1.5)
        nc.vector.scalar_tensor_tensor(wt[:], yps[:], 0.5, wt[:],
                                       op0=OP.mult, op1=OP.add)

        # ---- S = W^T.T @ X^T, chunk maxima, stream S to DRAM ----
        M = work.tile([128, 4, NCHUNK], F32)
        sdv = s_dram.rearrange("(mt p nt c) e -> mt nt p (c e)",
                               mt=4, p=128, nt=8, c=8)
        for mt in range(4):
            for nt in range(8):
                sp = psS.tile([128, 512], F32)
                _l = wt[:, mt * 128:(mt + 1) * 128]
                _r = xt[:, nt * 512:(nt + 1) * 512]
                if MM_F32R:
                    _l = _l.bitcast(mybir.dt.float32r)
                    _r = _r.bitcast(mybir.dt.float32r)
                nc.tensor.matmul(sp[:], _l, _r, start=True, stop=True)
                nc.vector.reduce_max(M[:, mt, nt * 8:(nt + 1) * 8],
                                     sp[:].rearrange("p (c e) -> p c e", e=CH),
                                     axis=X_AX)
                stg = sstg.tile([128, 512], F32)
                nc.scalar.copy(stg[:], sp[:])
                nc.sync.dma_start(sdv[mt, nt], stg[:])

        # ---- chunk selection: tau = 33rd-largest chunk max ----
        msort = work.tile([128, 4, NCHUNK], F32)
        nc.vector.tensor_copy(msort[:], M[:])
        # sort the two 32-halves of each row-group independently...
        _bitonic_sort_desc(nc, work, msort[:].rearrange("p a (h n) -> p (a h) n",
                                                        h=2))
        # ...then the 33rd largest of the union = max over the rejected half
        # of the bitonic merge: min(A_i, revB_i).
        smin = work.tile([128, 4, NCHUNK // 2], F32)
        nc.vector.tensor_tensor(smin[:], msort[:, :, :NCHUNK // 2],
                                msort[:, :, NCHUNK // 2:][:, :, ::-1],
                                op=OP.min)
        tau = work.tile([128, 4], F32)
        nc.vector.reduce_max(tau[:], smin[:], axis=X_AX)
        mask = work.tile([128, 4, NCHUNK], F32)
        nc.vector.tensor_tensor(mask[:], M[:],
                                tau[:].broadcast_to((128, 4, NCHUNK)),
                                op=OP.is_ge)
        key = work.tile([128, 4, NCHUNK], F32)
        nc.vector.scalar_tensor_tensor(key[:], mask[:], 128.0, dio_t[:],
                                       op0=OP.mult, op1=OP.add)
        _bitonic_sort_desc(nc, work, key[:])
        nc.sync.dma_start(out_key.rearrange("(g p) k -> p g k", p=128), key[:])

        # ---- gather indices: idx = rowbase+191 - key (cands), 0 (pads) ----
        idxf = work.tile([128, 4, NSLOT], F32)
        nc.vector.tensor_tensor(idxf[:], rowb_t[:].broadcast_to((128, 4, NSLOT)),
                                key[:, :, :NSLOT], op=OP.subtract)
        gem = work.tile([128, 4, NSLOT], F32)
        nc.vector.tensor_scalar(gem[:], key[:, :, :NSLOT], 128.0, None,
                                op0=OP.is_ge)
        nc.vector.tensor_tensor(idxf[:], idxf[:], gem[:], op=OP.mult)
        # ---- gather candidate chunks: one [P,1]-indexed indirect DMA per
        # slot (multi-index indirect DMA and dma_gather fail on HW) ----
        idxi = work.tile([128, 4, NSLOT], I32)
        nc.vector.tensor_copy(idxi[:], idxf[:])
        G = work.tile([128, 4, NSLOT, CH], F32)
        for g in range(4):
            for s in range(NSLOT):
                nc.gpsimd.indirect_dma_start(
                    G[:, g, s, :], None, s_dram,
                    IndirectOffsetOnAxis(ap=idxi[:, g, s:s + 1], axis=0),
                )
        nc.sync.dma_start(out_g.rearrange("(g p) (s e) -> p g s e", p=128, e=CH),
                          G[:])


_BASS_CACHE = {}
LAST_RES = None


def _get_nc():
    if "nc" not in _BASS_CACHE:
        _BASS_CACHE["nc"] = build_bass()
    return _BASS_CACHE["nc"]


def _host_consts():
    eye = np.eye(128, dtype=np.float32)
    dio3 = np.broadcast_to(
        (63.0 - np.arange(CH, dtype=np.float32))[None, None, :], (128, 4, CH)
    ).copy()
    p = np.arange(128, dtype=np.float32)[:, None]
    g = np.arange(4, dtype=np.float32)[None, :]
    rowb = ((g * 128 + p) * CH + 191.0).astype(np.float32)
    return eye, dio3, rowb


def _reference_row(r, X, adj):
    """Exact (reference-math) top-32 of one row, for rare fallbacks."""
    s = X[r] @ X.T + 0.5 * ((adj[r] @ X) @ X.T + (X[r] @ X.T))
    s[r] = -np.inf
    part = np.argpartition(-s, TOPK - 1)[:TOPK]
    order = np.lexsort((part, -s[part]))
    idx = part[order]
    return s[idx].astype(np.float32), idx.astype(np.int64)


def kernel(embeds, adj):
    embeds = np.asarray(embeds, dtype=np.float32)
    adj = np.asarray(adj, dtype=np.float32)
    nc = _get_nc()
    eye, dio3, rowb = _host_consts()
    adjT = np.ascontiguousarray(adj.T)

    in_maps = []
    for c in range(NC):
        in_maps.append({
            "emb": embeds,
            "adjt": np.ascontiguousarray(adjT[:, c * RPC:(c + 1) * RPC]),
            "embsel": np.ascontiguousarray(embeds[c * RPC:(c + 1) * RPC]),
            "eye": eye,
            "dio3": dio3,
            "rowb": rowb,
        })
    import os as _os
    _trace = bool(_os.environ.get("KERNEL_TRACE"))
    res = run_bass_kernel_spmd(nc, in_maps, list(range(NC)), trace=_trace)
    global LAST_RES
    LAST_RES = res

    topv = np.empty((N, TOPK), dtype=np.float32)
    topi = np.empty((N, TOPK), dtype=np.int64)
    fallback_rows = []
    e64 = np.arange(CH, dtype=np.int64)
    for c in range(NC):
        g = res.results[c]["out_g"].reshape(RPC, NSLOT, CH)
        keys = res.results[c]["out_key"]  # [RPC, 64] sorted desc
        key40 = keys[:, :NSLOT]
        cand = key40 >= 128.0
        cid = (191.0 - key40).astype(np.int64)  # chunk id (valid where cand)
        cid = np.clip(cid, 0, NCHUNK - 1)
        cols = cid[:, :, None] * CH + e64[None, None, :]
        grows = c * RPC + np.arange(RPC)
        valid = cand[:, :, None] & (cols != grows[:, None, None])
        v = np.where(valid, g, -np.inf).reshape(RPC, NSLOT * CH)
        cf = cols.reshape(RPC, NSLOT * CH)
        part = np.argpartition(-v, TOPK - 1, axis=1)[:, :TOPK]
        pv = np.take_along_axis(v, part, axis=1)
        pc = np.take_along_axis(cf, part, axis=1)
        for r in range(RPC):
            order = np.lexsort((pc[r], -pv[r]))
            topv[c * RPC + r] = pv[r][order]
            topi[c * RPC + r] = pc[r][order]
        # overflow: more than NSLOT candidate chunks -> exact host fallback
        over = np.nonzero(keys[:, NSLOT] >= 128.0)[0]
        fallback_rows.extend((c * RPC + r) for r in over)

    if fallback_rows:
        nrm = np.sqrt((embeds.astype(np.float64) ** 2).sum(1, keepdims=True))
        Xh = (embeds / np.maximum(nrm, 1e-12)).astype(np.float32)
        for r in fallback_rows:
            topv[r], topi[r] = _reference_row(r, Xh, adj)

    di = np.arange(N, dtype=np.int32)
    row_idx = np.repeat(di, TOPK)
    col_idx = topi.reshape(-1).astype(np.int32)
    indices = np.stack([row_idx, col_idx], axis=0)
    values = topv.reshape(-1)
    u = np.unique(topi).astype(np.int32)
    sampled = np.full(N, N, dtype=np.int32)
    sampled[:u.shape[0]] = u
    return (sampled, indices, values)


if __name__ == "__main__":
    rng = np.random.default_rng(0)
    e = rng.standard_normal((N, D)).astype(np.float32)
    a = (rng.random((N, N)) < 0.004).astype(np.float32)
    out = kernel(e, a)
    print([o.shape for o in out])


# revision 21
# speedup vs baseline: 1.0293x; 1.0293x over previous
"""Trainium2 Bass kernel for nn_AttentionSampling (gnn_message_passing).

Math: X = l2norm(embeds); S = X@X.T + 0.5*(adj+I)@(X@X.T), diag=-inf;
top-32 per row + unique of indices.

Factored form used on device:  S = W @ X^T  with  W = 1.5*X + 0.5*(adj@X)
(16x fewer FLOPs than the naive 137-GFLOP form; identical math up to fp
rounding).

Sharding: row-shard across 8 cores (512 rows each).  Each core:
  - normalizes X (all 4096 rows) + its own 512 rows
  - Y^T = X^T @ adjT_local  (PE, fp32)
  - W^T = 1.5*Xl^T + 0.5*Y^T
  - S tiles = W^T.T @ X^T   (PE, fp32), streamed to DRAM; per-64-chunk max
  - selects the top-33(+ties) chunks per row via an exact chunk-max
    threshold (bitonic sorts on the 64 chunk maxima), gathers those
    chunks with a per-row indirect DMA
  - outputs gathered candidate chunks + sorted chunk keys
Host: decodes candidates (a guaranteed exact superset of the top-32),
masks the diagonal, takes the exact top-32 per row, and builds the
reference's (sampled_nodes, indices, values) tuple.
"""

import sys

for _p in ("/opt/trn_rl_repo", "/root/.axon_site/_ro/trn_rl_repo"):
    if _p not in sys.path:
        sys.path.insert(0, _p)

from contextlib import ExitStack

import numpy as np

import concourse.bass as bass
import concourse.bacc as bacc
import concourse.mybir as mybir
import concourse.tile as tile
from concourse.bass import IndirectOffsetOnAxis
from concourse.bass_utils import run_bass_kernel_spmd

F32 = mybir.dt.float32
I32 = mybir.dt.int32
X_AX = mybir.AxisListType.X
OP = mybir.AluOpType

import os as _os
MM_F32R = bool(_os.environ.get("MM_F32R"))
N, D = 4096, 128
NC = 8            # cores
RPC = N // NC     # rows per core = 512
TOPK = 32
CH = 64           # chunk size
NCHUNK = N // CH  # 64 chunks per row
NSLOT = 34        # gathered candidate chunks per row (33 + tie margin)
NEG = -3.0e38


def _bitonic_sort_desc(nc, pool, buf):
    """In-place descending bitonic sort along the last axis of buf
    ([128, G, n] fp32 SBUF tile AP). n must be a power of 2."""
    p, g, n = buf.shape
    scratch = pool.tile([p, g, n // 2], F32, tag="bsort_scr")
    k = 2
    while k <= n:
        j = k // 2
        while j >= 1:
            # windows of 2j; element i pairs with i+j.
            nw = n // (2 * j)          # windows per row
            wk = max(k // (2 * j), 1)  # windows per direction block
            v = buf.rearrange("p a (b w x t) -> p a b w x t", w=wk, x=2, t=j)
            nb = v.shape[2]            # direction blocks (alternate desc/asc)
            for par, desc in ((0, True), (1, False)) if nb > 1 else ((0, True),):
                blk = v[:, :, par::2] if nb > 1 else v
                lo = blk[:, :, :, :, 0, :]
                hi = blk[:, :, :, :, 1, :]
                cnt = int(np.prod(lo.shape[2:]))
                scr = scratch[:, :, :cnt].rearrange(
                    "p a (b w t) -> p a b w t", w=lo.shape[3], t=j
                )
                if desc:
                    nc.vector.tensor_tensor(scr, lo, hi, op=OP.min)
                    nc.vector.tensor_tensor(lo, lo, hi, op=OP.max)
                else:
                    nc.vector.tensor_tensor(scr, lo, hi, op=OP.max)
                    nc.vector.tensor_tensor(lo, lo, hi, op=OP.min)
                nc.vector.tensor_copy(hi, scr)
            j //= 2
        k *= 2


def build_bass():
    nc = bacc.Bacc(None)
    emb = nc.dram_tensor("emb", [N, D], F32, kind="ExternalInput").ap()
    adjt = nc.dram_tensor("adjt", [N, RPC], F32, kind="ExternalInput").ap()
    embsel = nc.dram_tensor("embsel", [RPC, D], F32, kind="ExternalInput").ap()
    eye = nc.dram_tensor("eye", [128, 128], F32, kind="ExternalInput").ap()
    dio3 = nc.dram_tensor("dio3", [128, 4, CH], F32, kind="ExternalInput").ap()
    rowb = nc.dram_tensor("rowb", [128, 4], F32, kind="ExternalInput").ap()

    out_g = nc.dram_tensor("out_g", [RPC, NSLOT * CH], F32, kind="ExternalOutput").ap()
    out_key = nc.dram_tensor("out_key", [RPC, NCHUNK], F32, kind="ExternalOutput").ap()

    s_dram = nc.dram_tensor("s_scratch", [RPC * NCHUNK, CH], F32, kind="Internal").ap()
    didx = nc.dram_tensor("didx_scratch", [4 * NSLOT * 8 * 128], mybir.dt.int16,
                          kind="Internal").ap()

    with TileKernel(nc) as body:
        body(emb, adjt, embsel, eye, dio3, rowb, out_g, out_key, s_dram, didx)
    nc.finalize()
    return nc


class TileKernel:
    def __init__(self, nc):
        self.nc = nc
        self.stack = ExitStack()

    def __enter__(self):
        self.tc = self.stack.enter_context(tile.TileContext(self.nc))
        return self._body

    def __exit__(self, *exc):
        return self.stack.__exit__(*exc)

    def _body(self, emb, adjt, embsel, eye, dio3, rowb, out_g, out_key, s_dram,
              didx):
        nc, tc, ctx = self.nc, self.tc, self.stack
        const = ctx.enter_context(tc.tile_pool(name="const", bufs=1))
        work = ctx.enter_context(tc.tile_pool(name="work", bufs=1))
        adjp = ctx.enter_context(tc.tile_pool(name="adjp", bufs=3))
        sstg = ctx.enter_context(tc.tile_pool(name="sstg", bufs=4))
        psA = ctx.enter_context(tc.tile_pool(name="psA", bufs=2, space="PSUM"))
        psY = ctx.enter_context(tc.tile_pool(name="psY", bufs=1, space="PSUM"))
        psS = ctx.enter_context(tc.tile_pool(name="psS", bufs=4, space="PSUM"))

        eye_t = const.tile([128, 128], F32)
        nc.gpsimd.dma_start(eye_t[:], eye)
        dio_t = const.tile([128, 4, CH], F32)
        nc.gpsimd.dma_start(dio_t[:], dio3)
        rowb_t = const.tile([128, 4], F32)
        nc.gpsimd.dma_start(rowb_t[:], rowb)

        # ---- load embeds (rows r=t*128+p on partition p, group t) ----
        embt = work.tile([128, N // 128, D], F32)
        nc.sync.dma_start(embt[:], emb.rearrange("(t p) d -> p t d", p=128))
        eselt = work.tile([128, RPC // 128, D], F32)
        nc.sync.dma_start(eselt[:], embsel.rearrange("(g p) d -> p g d", p=128))

        # ---- normalize: X = emb / sqrt(sum(emb^2)) ----
        def normalize(src, nt):
            sq = work.tile([128, nt, D], F32, tag="sq")
            nc.scalar.activation(sq[:], src[:],
                                 mybir.ActivationFunctionType.Square)
            ss = work.tile([128, nt], F32, tag="ss")
            nc.vector.reduce_sum(ss[:], sq[:], axis=X_AX)
            nrm = work.tile([128, nt], F32, tag="nrm")
            nc.scalar.activation(nrm[:], ss[:], mybir.ActivationFunctionType.Sqrt)
            inv = work.tile([128, nt], F32, tag="inv")
            nc.vector.reciprocal(inv[:], nrm[:])
            xo = work.tile([128, nt, D], F32, tag="xnorm" + str(nt))
            nc.vector.tensor_tensor(xo[:], src[:], inv[:].broadcast_to((128, nt, D)),
                                    op=OP.mult)
            return xo

        X = normalize(embt, N // 128)        # [128, 32, 128] rows x d
        Xl = normalize(eselt, RPC // 128)    # [128, 4, 128] own rows x d

        # ---- transposes via PE: X_T [128 d, 4096 rows], Xl_T [128 d, 512] ----
        xt = work.tile([128, N], F32)
        for t in range(N // 128):
            ps = psA.tile([128, 128], F32)
            nc.tensor.transpose(ps[:], X[:, t, :], eye_t[:])
            nc.scalar.copy(xt[:, t * 128:(t + 1) * 128], ps[:])
        xlt = work.tile([128, RPC], F32)
        for g in range(RPC // 128):
            ps = psA.tile([128, 128], F32)
            nc.tensor.transpose(ps[:], Xl[:, g, :], eye_t[:])
            nc.scalar.copy(xlt[:, g * 128:(g + 1) * 128], ps[:])

        # ---- Y^T = X^T @ adjT_local : accumulate over 32 k-chunks ----
        yps = psY.tile([128, RPC], F32)
        for kc in range(N // 128):
            at = adjp.tile([128, RPC], F32)
            nc.sync.dma_start(at[:], adjt[kc * 128:(kc + 1) * 128, :])
            _l, _r = X[:, kc, :], at[:]
            if MM_F32R:
                _l, _r = _l.bitcast(mybir.dt.float32r), _r.bitcast(mybir.dt.float32r)
            nc.tensor.matmul(yps[:], _l, _r,
                             start=(kc == 0), stop=(kc == N // 128 - 1))

        # ---- W^T = 1.5*Xl^T + 0.5*Y^T ----
        wt = work.tile([128, RPC], F32)
        nc.vector.tensor_scalar_mul(wt[:], xlt[:], 1.5)
        nc.vector.scalar_tensor_tensor(wt[:], yps[:], 0.5, wt[:],
                                       op0=OP.mult, op1=OP.add)

        # ---- S = W^T.T @ X^T, chunk maxima, stream S to DRAM ----
        M = work.tile([128, 4, NCHUNK], F32)
        sdv = s_dram.rearrange("(mt p nt c) e -> mt nt p (c e)",
                               mt=4, p=128, nt=8, c=8)
        for mt in range(4):
            for nt in range(8):
                sp = psS.tile([128, 512], F32)
                _l = wt[:, mt * 128:(mt + 1) * 128]
                _r = xt[:, nt * 512:(nt + 1) * 512]
                if MM_F32R:
                    _l = _l.bitcast(mybir.dt.float32r)
                    _r = _r.bitcast(mybir.dt.float32r)
                nc.tensor.matmul(sp[:], _l, _r, start=True, stop=True)
                nc.vector.reduce_max(M[:, mt, nt * 8:(nt + 1) * 8],
                                     sp[:].rearrange("p (c e) -> p c e", e=CH),
                                     axis=X_AX)
                stg = sstg.tile([128, 512], F32)
                nc.scalar.copy(stg[:], sp[:])
                nc.sync.dma_start(sdv[mt, nt], stg[:])

        # ---- chunk selection: tau = 33rd-largest chunk max ----
        msort = work.tile([128, 4, NCHUNK], F32)
        nc.vector.tensor_copy(msort[:], M[:])
        # sort the two 32-halves of each row-group independently...
        _bitonic_sort_desc(nc, work, msort[:].rearrange("p a (h n) -> p (a h) n",
                                                        h=2))
        # ...then the 33rd largest of the union = max over the rejected half
        # of the bitonic merge: min(A_i, revB_i).
        smin = work.tile([128, 4, NCHUNK // 2], F32)
        nc.vector.tensor_tensor(smin[:], msort[:, :, :NCHUNK // 2],
                                msort[:, :, NCHUNK // 2:][:, :, ::-1],
                                op=OP.min)
        tau = work.tile([128, 4], F32)
        nc.vector.reduce_max(tau[:], smin[:], axis=X_AX)
        mask = work.tile([128, 4, NCHUNK], F32)
        nc.vector.tensor_tensor(mask[:], M[:],
                                tau[:].broadcast_to((128, 4, NCHUNK)),
                                op=OP.is_ge)
        key = work.tile([128, 4, NCHUNK], F32)
        nc.vector.scalar_tensor_tensor(key[:], mask[:], 128.0, dio_t[:],
                                       op0=OP.mult, op1=OP.add)
        _bitonic_sort_desc(nc, work, key[:])
        nc.sync.dma_start(out_key.rearrange("(g p) k -> p g k", p=128), key[:])

        # ---- gather indices: idx = rowbase+191 - key (cands), 0 (pads) ----
        idxf = work.tile([128, 4, NSLOT], F32)
        nc.vector.tensor_tensor(idxf[:], rowb_t[:].broadcast_to((128, 4, NSLOT)),
                                key[:, :, :NSLOT], op=OP.subtract)
        gem = work.tile([128, 4, NSLOT], F32)
        nc.vector.tensor_scalar(gem[:], key[:, :, :NSLOT], 128.0, None,
                                op0=OP.is_ge)
        nc.vector.tensor_tensor(idxf[:], idxf[:], gem[:], op=OP.mult)
        # ---- gather candidate chunks: one [P,1]-indexed indirect DMA per
        # slot (multi-index indirect DMA and dma_gather fail on HW) ----
        idxi = work.tile([128, 4, NSLOT], I32)
        nc.vector.tensor_copy(idxi[:], idxf[:])
        G = work.tile([128, 4, NSLOT, CH], F32)
        for g in range(4):
            for s in range(NSLOT):
                nc.gpsimd.indirect_dma_start(
                    G[:, g, s, :], None, s_dram,
                    IndirectOffsetOnAxis(ap=idxi[:, g, s:s + 1], axis=0),
                )
        nc.sync.dma_start(out_g.rearrange("(g p) (s e) -> p g s e", p=128, e=CH),
                          G[:])


_BASS_CACHE = {}
LAST_RES = None


def _get_nc():
    if "nc" not in _BASS_CACHE:
        _BASS_CACHE["nc"] = build_bass()
    return _BASS_CACHE["nc"]


def _host_consts():
    eye = np.eye(128, dtype=np.float32)
    dio3 = np.broadcast_to(
        (63.0 - np.arange(CH, dtype=np.float32))[None, None, :], (128, 4, CH)
    ).copy()
    p = np.arange(128, dtype=np.float32)[:, None]
    g = np.arange(4, dtype=np.float32)[None, :]
    rowb = ((g * 128 + p) * CH + 191.0).astype(np.float32)
    return eye, dio3, rowb


def _reference_row(r, X, adj):
    """Exact (reference-math) top-32 of one row, for rare fallbacks."""
    s = X[r] @ X.T + 0.5 * ((adj[r] @ X) @ X.T + (X[r] @ X.T))
    s[r] = -np.inf
    part = np.argpartition(-s, TOPK - 1)[:TOPK]
    order = np.lexsort((part, -s[part]))
    idx = part[order]
    return s[idx].astype(np.float32), idx.astype(np.int64)


def kernel(embeds, adj):
    embeds = np.asarray(embeds, dtype=np.float32)
    adj = np.asarray(adj, dtype=np.float32)
    nc = _get_nc()
    eye, dio3, rowb = _host_consts()
    adjT = np.ascontiguousarray(adj.T)

    in_maps = []
    for c in range(NC):
        in_maps.append({
            "emb": embeds,
            "adjt": np.ascontiguousarray(adjT[:, c * RPC:(c + 1) * RPC]),
            "embsel": np.ascontiguousarray(embeds[c * RPC:(c + 1) * RPC]),
            "eye": eye,
            "dio3": dio3,
            "rowb": rowb,
        })
    import os as _os
    _trace = bool(_os.environ.get("KERNEL_TRACE"))
    res = run_bass_kernel_spmd(nc, in_maps, list(range(NC)), trace=_trace)
    global LAST_RES
    LAST_RES = res

    topv = np.empty((N, TOPK), dtype=np.float32)
    topi = np.empty((N, TOPK), dtype=np.int64)
    fallback_rows = []
    e64 = np.arange(CH, dtype=np.int64)
    for c in range(NC):
        g = res.results[c]["out_g"].reshape(RPC, NSLOT, CH)
        keys = res.results[c]["out_key"]  # [RPC, 64] sorted desc
        key40 = keys[:, :NSLOT]
        cand = key40 >= 128.0
        cid = (191.0 - key40).astype(np.int64)  # chunk id (valid where cand)
        cid = np.clip(cid, 0, NCHUNK - 1)
        cols = cid[:, :, None] * CH + e64[None, None, :]
        grows = c * RPC + np.arange(RPC)
        valid = cand[:, :, None] & (cols != grows[:, None, None])
        v = np.where(valid, g, -np.inf).reshape(RPC, NSLOT * CH)
        cf = cols.reshape(RPC, NSLOT * CH)
        part = np.argpartition(-v, TOPK - 1, axis=1)[:, :TOPK]
        pv = np.take_along_axis(v, part, axis=1)
        pc = np.take_along_axis(cf, part, axis=1)
        for r in range(RPC):
            order = np.lexsort((pc[r], -pv[r]))
            topv[c * RPC + r] = pv[r][order]
            topi[c * RPC + r] = pc[r][order]
        # overflow: more than NSLOT candidate chunks -> exact host fallback
        over = np.nonzero(keys[:, NSLOT] >= 128.0)[0]
        fallback_rows.extend((c * RPC + r) for r in over)

    if fallback_rows:
        nrm = np.sqrt((embeds.astype(np.float64) ** 2).sum(1, keepdims=True))
        Xh = (embeds / np.maximum(nrm, 1e-12)).astype(np.float32)
        for r in fallback_rows:
            topv[r], topi[r] = _reference_row(r, Xh, adj)

    di = np.arange(N, dtype=np.int32)
    row_idx = np.repeat(di, TOPK)
    col_idx = topi.reshape(-1).astype(np.int32)
    indices = np.stack([row_idx, col_idx], axis=0)
    values = topv.reshape(-1)
    u = np.unique(topi).astype(np.int32)
    sampled = np.full(N, N, dtype=np.int32)
    sampled[:u.shape[0]] = u
    return (sampled, indices, values)


if __name__ == "__main__":
    rng = np.random.default_rng(0)
    e = rng.standard_normal((N, D)).astype(np.float32)
    a = (rng.random((N, N)) < 0.004).astype(np.float32)
    out = kernel(e, a)
    print([o.shape for o in out])
